# revision 8
# baseline (speedup 1.0000x reference)
"""Trainium2 Bass kernel for nn_MlpwithSOMModule (retrieval_knn).

Reference computation, per (b, k) pair with L=128, D=768:
    ctx, ent = context[b,k,0], context[b,k,1]          # [L, D] each
    S        = ctx @ ent.T                             # [L, L]
    idx      = argmax_m S[l, m]
    best     = ent[idx]                                # [L, D]
    out[l]   = f(ctx[l]) + f(best[l])                  # f = 3-layer MLP -> scalar

Key restructuring: instead of gathering 768-wide rows, compute the scalar MLP
output f for ALL ctx rows and ALL ent rows (same FLOP count: 2L rows either
way), then resolve the gather as a one-hot weighted sum of scalars:
    out[l] = f(ctx[l]) + sum_m onehot[l,m] * f(ent[m])
with onehot = (S == rowmax(S)).  Ties are measure-zero for random data
(validated: zero ties on the actual inputs, max abs err 2.7e-7 vs reference).

All matmuls contract over D, so activations live in transposed layout
[d_partition, row_free].  Raw inputs are transposed once on the PE
(6x [128,128] tile transposes per [128,768] operand); every later layer
*produces* its output already transposed (H1T = W1.T-chunks @ XT etc.), so no
further transposes are needed.

Precision (HW-measured): scores run plain fp32 matmuls (exact, ~1.6e-7 rel) so
the argmax matches the fp32 reference; the MLP runs float32r (fp32 fast path,
1 cycle/row at moving dim >= 256, ~1.6e-4 rel — far inside tolerance).  All
float32r matmul operands must be explicitly rounded by their producer ops
(walrus verifier requirement), so the transposed activations are evacuated
from PSUM twice: once as fp32 for scores, once as f32r for the MLP; MLP layer
outputs are written as f32r directly by their ReLU evacuation ops.

Sharding: data-parallel over the 256 (b,k) pairs -> 32 per NeuronCore, weights
replicated.  Two pairs are processed per inner iteration so the MLP moving
dimension is 512 (= PSUM bank capacity in fp32).
"""

from contextlib import ExitStack

import numpy as np

import concourse.bacc as bacc
import concourse.mybir as mybir
import concourse.tile as tile
from concourse.bass_utils import run_bass_kernel_spmd
from concourse.masks import make_identity

B, K, L, D = 4, 64, 128, 768
N_CORES = 8
BK = B * K                      # 256 (b,k) pairs total
BK_PER_CORE = BK // N_CORES     # 32
PAIR = 2                        # pairs per inner iteration (moving dim 512)
DC = D // 128                   # 6 contraction chunks
NCOL = PAIR * 2 * 128           # 512 columns per iteration

F32 = mybir.dt.float32
F32R = mybir.dt.float32r


def build_kernel(n_bk: int = BK_PER_CORE):
    assert n_bk % PAIR == 0
    nc = bacc.Bacc("TRN2", target_bir_lowering=False)

    x = nc.declare_dram_parameter("x", [n_bk, 2, L, D], F32, isOutput=False)
    w1 = nc.declare_dram_parameter("w1", [D, D], F32, isOutput=False)
    b1 = nc.declare_dram_parameter("b1", [D], F32, isOutput=False)
    w2 = nc.declare_dram_parameter("w2", [D, D], F32, isOutput=False)
    b2 = nc.declare_dram_parameter("b2", [D], F32, isOutput=False)
    w3 = nc.declare_dram_parameter("w3", [D, 1], F32, isOutput=False)
    b3 = nc.declare_dram_parameter("b3", [1], F32, isOutput=False)
    out = nc.declare_dram_parameter("out", [n_bk, L], F32, isOutput=True)

    with tile.TileContext(nc) as tc:
        with ExitStack() as ctx:
            _emit(ctx, tc, n_bk, x, w1, b1, w2, b2, w3, b3, out)
    nc.compile()
    return nc


def _emit(ctx, tc, n_bk, x, w1, b1, w2, b2, w3, b3, out):
    nc = tc.nc
    AF = mybir.ActivationFunctionType
    ALU = mybir.AluOpType

    consts = ctx.enter_context(tc.tile_pool(name="consts", bufs=1))
    raw = ctx.enter_context(tc.tile_pool(name="raw", bufs=3))
    xt = ctx.enter_context(tc.tile_pool(name="xt", bufs=2))
    hp = ctx.enter_context(tc.tile_pool(name="hp", bufs=2))
    small = ctx.enter_context(tc.tile_pool(name="small", bufs=4))
    scratch = ctx.enter_context(tc.tile_pool(name="scratch", bufs=2))
    pmm = ctx.enter_context(tc.tile_pool(name="pmm", bufs=2, space="PSUM"))
    p128 = ctx.enter_context(tc.tile_pool(name="p128", bufs=4, space="PSUM"))
    posm = ctx.enter_context(tc.tile_pool(name="posm", bufs=1, space="PSUM"))

    # ---- constants / weights (loaded once) ----
    w1_sb = consts.tile([128, DC, D], F32)
    nc.sync.dma_start(out=w1_sb, in_=w1.rearrange("(c p) j -> p c j", p=128))
    w2_sb = consts.tile([128, DC, D], F32)
    nc.sync.dma_start(out=w2_sb, in_=w2.rearrange("(c p) j -> p c j", p=128))
    w3_sb = consts.tile([128, DC], F32)
    nc.sync.dma_start(out=w3_sb, in_=w3.rearrange("(c p) one -> p (c one)", p=128))
    b1_sb = consts.tile([128, DC], F32)
    nc.sync.dma_start(out=b1_sb, in_=b1.rearrange("(c p) -> p c", p=128))
    b2_sb = consts.tile([128, DC], F32)
    nc.sync.dma_start(out=b2_sb, in_=b2.rearrange("(c p) -> p c", p=128))
    b3_sb = consts.tile([1, 1], F32)
    nc.sync.dma_start(out=b3_sb, in_=b3[:].unsqueeze(0))

    # f32r-rounded weight copies for the MLP matmuls
    w1_r = consts.tile([128, DC, D], F32R)
    nc.vector.tensor_copy(w1_r, w1_sb)
    w2_r = consts.tile([128, DC, D], F32R)
    nc.vector.tensor_copy(w2_r, w2_sb)
    w3_r = consts.tile([128, DC], F32R)
    nc.vector.tensor_copy(w3_r, w3_sb)

    ones_f = consts.tile([1, 128], F32)
    nc.vector.memset(ones_f, 1.0)
    ones_r = consts.tile([1, 128], F32R)
    nc.vector.tensor_copy(ones_r, ones_f)
    ident = consts.tile([128, 128], F32)
    make_identity(nc, ident)

    res_all = consts.tile([128, n_bk], F32)

    n_iter = n_bk // PAIR
    for it in range(n_iter):
        # ---- load raw pairs: [l_part, pair, which(ctx/ent), d] ----
        raw_t = raw.tile([128, PAIR, 2, D], F32)
        nc.sync.dma_start(
            out=raw_t,
            in_=x[it * PAIR : (it + 1) * PAIR].transpose([2, 0, 1, 3]),
        )

        # ---- transpose to XT: [d_part, chunk, col] with col = p*256 + w*128 + l
        # evacuated twice: fp32 copy for the score matmuls, f32r for MLP L1
        xt_t = xt.tile([128, DC, NCOL], F32, tag="xt")
        xt_r = xt.tile([128, DC, NCOL], F32R, tag="xtr")
        for c in range(DC):
            for q in range(PAIR * 2):  # q = p*2 + w
                tr_ps = p128.tile([128, 128], F32, tag="p128")
                nc.tensor.transpose(
                    tr_ps, raw_t[:, q // 2, q % 2, c * 128 : (c + 1) * 128], ident
                )
                nc.vector.tensor_copy(xt_t[:, c, q * 128 : (q + 1) * 128], tr_ps)
                nc.vector.tensor_copy(xt_r[:, c, q * 128 : (q + 1) * 128], tr_ps)

        # ---- scores + one-hot per pair (plain fp32 for exact argmax) ----
        onehots = []
        for p in range(PAIR):
            s_ps = p128.tile([128, 128], F32, tag="p128")
            for c in range(DC):
                nc.tensor.matmul(
                    s_ps,
                    lhsT=xt_t[:, c, (2 * p) * 128 : (2 * p + 1) * 128],
                    rhs=xt_t[:, c, (2 * p + 1) * 128 : (2 * p + 2) * 128],
                    start=(c == 0),
                    stop=(c == DC - 1),
                )
            rm = small.tile([128, 1], F32)
            nc.vector.reduce_max(rm, s_ps, axis=mybir.AxisListType.X)
            oh = scratch.tile([128, 128], F32, tag="oh")
            nc.vector.tensor_scalar(
                out=oh, in0=s_ps, scalar1=rm, scalar2=None, op0=ALU.is_equal
            )
            onehots.append(oh)

        # ---- MLP layers 1+2 (transposed: out[j, col] chunks), f32r ----
        def mlp_layer(src_t, w_r, b_sb):
            dst_t = hp.tile([128, DC, NCOL], F32R, tag="h")
            for j in range(DC):
                mm = pmm.tile([128, NCOL], F32, tag="mm")
                for c in range(DC):
                    nc.tensor.matmul(
                        mm,
                        lhsT=w_r[:, c, j * 128 : (j + 1) * 128],
                        rhs=src_t[:, c, :],
                        start=(c == 0),
                        stop=(c == DC - 1),
                    )
                # ReLU + per-partition bias, written rounded to f32r (ACT)
                nc.scalar.activation(
                    out=dst_t[:, j, :], in_=mm, func=AF.Relu, bias=b_sb[:, j : j + 1]
                )
            return dst_t

        h1_t = mlp_layer(xt_r, w1_r, b1_sb)
        h2_t = mlp_layer(h1_t, w2_r, b2_sb)

        # ---- layer 3: o_row[0, col] = sum_j W3[j] * H2T[j, col] (+ b3) ----
        orow = posm.tile([1, NCOL], F32, tag="orow")
        for c in range(DC):
            nc.tensor.matmul(
                orow,
                lhsT=w3_r[:, c : c + 1],
                rhs=h2_t[:, c, :],
                start=(c == 0),
                stop=(c == DC - 1),
            )
        o_sb = small.tile([1, NCOL], F32R, tag="osb")
        nc.vector.tensor_scalar(
            out=o_sb, in0=orow, scalar1=b3_sb[0:1, 0:1], scalar2=None, op0=ALU.add
        )

        # ---- broadcast o to all partitions: obc[p, col] = o_sb[col] ----
        obc = posm.tile([128, NCOL], F32, tag="obc")
        nc.tensor.matmul(obc, lhsT=ones_r, rhs=o_sb, start=True, stop=True)

        # ---- result: res[l] = o_ctx[l] + sum_m onehot[l,m] * o_ent[m] ----
        # (tensor_tensor_reduce faults on this HW path, so mult + reduce_sum)
        for p in range(PAIR):
            prod = scratch.tile([128, 128], F32, tag="prod")
            nc.vector.tensor_mul(
                prod, onehots[p], obc[:, (2 * p + 1) * 128 : (2 * p + 2) * 128]
            )
            rent = small.tile([128, 1], F32)
            nc.vector.reduce_sum(rent, prod, axis=mybir.AxisListType.X)
            prod2 = scratch.tile([128, 128], F32, tag="prod")
            nc.vector.tensor_mul(
                prod2, ident, obc[:, (2 * p) * 128 : (2 * p + 1) * 128]
            )
            rctx = small.tile([128, 1], F32)
            nc.vector.reduce_sum(rctx, prod2, axis=mybir.AxisListType.X)
            nc.vector.tensor_add(
                res_all[:, it * PAIR + p : it * PAIR + p + 1], rent, rctx
            )

    # ---- store: res_all [l_part, bk] -> out[bk, l] ----
    nc.sync.dma_start(out=out.transpose([1, 0]), in_=res_all)


_NC_CACHE = {}


def _get_nc(n_bk):
    if n_bk not in _NC_CACHE:
        _NC_CACHE[n_bk] = build_kernel(n_bk)
    return _NC_CACHE[n_bk]


def run(inputs, trace=False):
    context = np.ascontiguousarray(np.asarray(inputs["context"], dtype=np.float32))
    xs = context.reshape(BK, 2, L, D)
    shared = {
        "w1": np.ascontiguousarray(np.asarray(inputs["W1"], dtype=np.float32)),
        "b1": np.ascontiguousarray(np.asarray(inputs["b1"], dtype=np.float32)),
        "w2": np.ascontiguousarray(np.asarray(inputs["W2"], dtype=np.float32)),
        "b2": np.ascontiguousarray(np.asarray(inputs["b2"], dtype=np.float32)),
        "w3": np.ascontiguousarray(np.asarray(inputs["W3"], dtype=np.float32)),
        "b3": np.ascontiguousarray(np.asarray(inputs["b3"], dtype=np.float32)),
    }
    in_maps = [
        {"x": np.ascontiguousarray(xs[c * BK_PER_CORE : (c + 1) * BK_PER_CORE]), **shared}
        for c in range(N_CORES)
    ]
    nc = _get_nc(BK_PER_CORE)
    res = run_bass_kernel_spmd(nc, in_maps, list(range(N_CORES)), trace=trace)
    outs = [m["out"] for m in res.results]
    full = np.concatenate(outs, axis=0).reshape(B, K, L).astype(np.float32)
    return full, res


def kernel(**inputs) -> np.ndarray:
    full, _ = run(inputs, trace=False)
    return full
